# revision 15
# baseline (speedup 1.0000x reference)
"""GraphSAGE (3x SAGEConv mean-aggr + concat + global_add_pool + linear) on 8 trn2 cores.

Strategy (data-parallel over dst nodes):
- Host: dst-shard nodes 5000/core; per core, permute its nodes by (graph, -deg);
  build padded per-tile gather index grids (128 dst x K slots, K capped at 24 with
  overflow rounds); remap all src ids into the permuted-global layout with a
  5008-row per-core stride whose last 8 rows are zero (dummy slot target).
- Device: per layer, indirect-DMA gather of neighbor rows (128 rows / instr),
  DVE halving-tree segment-sum, inv-degree scale, PE transpose to feature-major,
  PE matmuls agg@Wl + h@Wr accumulated in PSUM, fused bias+relu on ACT,
  PE transpose back, pooling via one-hot matmul into PSUM accumulator,
  AllGather of the node-feature slice between layers, final AllReduce of pooled
  partials + tiny linear, identical on every core.
"""
import os
import shutil
import sys

import numpy as np

sys.path.insert(0, "/opt/trn_rl_repo")

N, E, F, G, C = 40000, 640000, 128, 64, 8
S = N // C            # 5000 real nodes per core
SP = S + 8            # padded per-core slice rows (last 8 rows zero)
NT = 40               # dst tiles of 128 (40*128 = 5120 >= 5000)
KCAP = 24
DUMMY = SP - 1        # row 5007: zero row of core 0's slice
P = 128


def _host_prep(x, edge_index, batch):
    src, dst = np.asarray(edge_index[0]), np.asarray(edge_index[1])
    batch = np.asarray(batch)
    deg = np.bincount(dst, minlength=N).astype(np.int64)
    inv = (1.0 / np.maximum(deg, 1)).astype(np.float32)

    # per-core permutation: sort by (graph, -deg) within the core's node range
    pg = np.empty(N, dtype=np.int64)          # node -> permuted-global id
    perm_nodes = np.empty((C, S), dtype=np.int64)  # [c, r] -> node
    for c in range(C):
        nodes = np.arange(c * S, (c + 1) * S)
        order = np.lexsort((-deg[nodes], batch[nodes]))
        pn = nodes[order]
        perm_nodes[c] = pn
        pg[pn] = c * SP + np.arange(S)

    xp = np.zeros((C * SP, F), dtype=np.float32)
    for c in range(C):
        xp[c * SP: c * SP + S] = x[perm_nodes[c]]

    # per-core transposed x slice [F, NT*128]
    xsliceT = np.zeros((C, F, NT * P), dtype=np.float32)
    for c in range(C):
        xsliceT[c, :, :S] = xp[c * SP: c * SP + S].T

    # group edges by dst core / local permuted rank
    src_pg = pg[src]
    dst_core = dst // S
    # local rank of dst within its core's permutation
    local_rank = (pg[dst] % SP).astype(np.int64)

    # CSR per core over local ranks
    idx_cols = [[] for _ in range(C)]   # list of [128, K] int32 arrays per core
    schedule = None
    percore_sched = []
    percore_lists = []
    for c in range(C):
        m = dst_core == c
        lr = local_rank[m]
        sp = src_pg[m]
        order = np.argsort(lr, kind="stable")
        lr, sp = lr[order], sp[order]
        starts = np.searchsorted(lr, np.arange(S))
        ends = np.searchsorted(lr, np.arange(S) + 1)
        percore_lists.append((starts, ends, sp))
        sched_c = []
        for t in range(NT):
            r0 = t * P
            degs = np.zeros(P, dtype=np.int64)
            nreal = min(P, S - r0) if r0 < S else 0
            if nreal > 0:
                degs[:nreal] = ends[r0:r0 + nreal] - starts[r0:r0 + nreal]
            maxd = int(degs.max()) if nreal > 0 else 0
            rounds = []
            off = 0
            while True:
                k = min(KCAP, maxd - off)
                if off == 0:
                    k = max(k, 1)   # always at least one round
                if k <= 0:
                    break
                rounds.append(k)
                off += k
                if off >= maxd:
                    break
            sched_c.append(rounds)
        percore_sched.append(sched_c)

    # uniform schedule across cores: per tile, round_j K = max over cores
    schedule = []   # list of (t, j, K)
    for t in range(NT):
        nr = max(len(percore_sched[c][t]) for c in range(C))
        for j in range(nr):
            k = max((percore_sched[c][t][j] if j < len(percore_sched[c][t]) else 0)
                    for c in range(C))
            k = int(np.ceil(k / 2) * 2)  # even K
            schedule.append((t, j, k))
    totk = sum(k for _, _, k in schedule)

    idx_arr = np.full((C, P, totk), DUMMY, dtype=np.int32)
    for c in range(C):
        starts, ends, sp = percore_lists[c]
        col = 0
        for (t, j, k) in schedule:
            r0 = t * P
            nreal = min(P, S - r0) if r0 < S else 0
            for p in range(nreal):
                s0, e0 = starts[r0 + p], ends[r0 + p]
                lo = s0 + j * KCAP
                hi = min(lo + k, e0)
                if hi > lo:
                    idx_arr[c, p, col:col + (hi - lo)] = sp[lo:hi]
            col += k

    invdeg = np.ones((C, P, NT), dtype=np.float32)
    Bmat = np.zeros((C, P, NT, G), dtype=np.float32)
    for c in range(C):
        for t in range(NT):
            r0 = t * P
            nreal = min(P, S - r0) if r0 < S else 0
            if nreal <= 0:
                continue
            nodes = perm_nodes[c][r0:r0 + nreal]
            invdeg[c, :nreal, t] = inv[nodes]
            Bmat[c, np.arange(nreal), t, batch[nodes]] = 1.0
    Bmat = Bmat.reshape(C, P, NT * G)

    return xp, xsliceT, idx_arr, invdeg, Bmat, schedule, totk


def _build_program(schedule, totk):
    import concourse.bass as bass
    import concourse.tile as tile
    from concourse import bacc, mybir
    from concourse.masks import make_identity

    f32 = mybir.dt.float32
    nc = bacc.Bacc("TRN2", target_bir_lowering=False, debug=False, num_devices=C)

    # I/O
    xperm = nc.dram_tensor("xperm", [C * SP, F], f32, kind="ExternalInput")
    xsliceT = nc.dram_tensor("xsliceT", [F, NT * P], f32, kind="ExternalInput")
    idxs = nc.dram_tensor("idxs", [P, totk], mybir.dt.int32, kind="ExternalInput")
    invdeg = nc.dram_tensor("invdeg", [P, NT], f32, kind="ExternalInput")
    Bmat = nc.dram_tensor("Bmat", [P, NT * G], f32, kind="ExternalInput")
    Wls, Wrs, bs = [], [], []
    for l in (1, 2, 3):
        Wls.append(nc.dram_tensor(f"W{l}l", [F, F], f32, kind="ExternalInput"))
        Wrs.append(nc.dram_tensor(f"W{l}r", [F, F], f32, kind="ExternalInput"))
        bs.append(nc.dram_tensor(f"b{l}", [F, 1], f32, kind="ExternalInput"))
    WlinT = nc.dram_tensor("WlinT", [F, 3 * F], f32, kind="ExternalInput")
    blin = nc.dram_tensor("blin", [F, 1], f32, kind="ExternalInput")
    out = nc.dram_tensor("out", [G, F], f32, kind="ExternalOutput")
    dbg = nc.dram_tensor("dbg", [SP, F], f32, kind="ExternalOutput")

    # internals
    slice_b = [nc.dram_tensor(f"slice_b{l}", [SP, F], f32) for l in (1, 2)]
    hg = [nc.dram_tensor(f"hg{l}", [C * SP, F], f32) for l in (2, 3)]
    pool_b = nc.dram_tensor("pool_b", [3 * G, F], f32)
    pool_r = nc.dram_tensor("pool_r", [3 * G, F], f32)

    AOT = mybir.AluOpType
    AFT = mybir.ActivationFunctionType
    CH = 512  # matmul free-dim chunk

    with tile.TileContext(nc) as tc:
        with tc.tile_pool(name="persist", bufs=1) as pp, \
             tc.tile_pool(name="gat", bufs=5) as gp, \
             tc.tile_pool(name="hrow", bufs=3) as hp, \
             tc.tile_pool(name="hT", bufs=2) as hTp, \
             tc.tile_pool(name="agg", bufs=1) as aggp, \
             tc.tile_pool(name="ps_t", bufs=2, space="PSUM") as pst, \
             tc.tile_pool(name="ps_mm", bufs=2, space="PSUM") as psm, \
             tc.tile_pool(name="ps_small", bufs=1, space="PSUM") as pss:

            ident = pp.tile([P, P], f32)
            make_identity(nc, ident[:])

            idx_sb = pp.tile([P, totk], mybir.dt.int32)
            nc.sync.dma_start(out=idx_sb[:], in_=idxs[:])
            invd_sb = pp.tile([P, NT], f32)
            nc.sync.dma_start(out=invd_sb[:], in_=invdeg[:])
            B_sb = pp.tile([P, NT * G], f32)
            nc.sync.dma_start(out=B_sb[:], in_=Bmat[:])
            W_sb = []
            for l in range(3):
                wl = pp.tile([F, F], f32, tag=f"wl{l}")
                nc.sync.dma_start(out=wl[:], in_=Wls[l][:])
                wr = pp.tile([F, F], f32, tag=f"wr{l}")
                nc.sync.dma_start(out=wr[:], in_=Wrs[l][:])
                bb = pp.tile([F, 1], f32, tag=f"b{l}")
                nc.sync.dma_start(out=bb[:], in_=bs[l][:])
                W_sb.append((wl, bb, wr))
            wlin_sb = pp.tile([F, 3 * F], f32)
            nc.sync.dma_start(out=wlin_sb[:], in_=WlinT[:])
            blin_sb = pp.tile([F, 1], f32)
            nc.sync.dma_start(out=blin_sb[:], in_=blin[:])

            zero8 = pp.tile([8, F], f32)
            nc.vector.memset(zero8[:], 0.0)
            for sb in slice_b:
                nc.sync.dma_start(out=sb[S:SP, :], in_=zero8[:])

            hT_cur = pp.tile([F, NT * P], f32, tag="hT0")
            nc.sync.dma_start(out=hT_cur[:], in_=xsliceT[:])

            pooled = pp.tile([G, 3 * F], f32)

            tables = [xperm, hg[0], hg[1]]
            for l in range(3):
                table = tables[l]
                Wl_sb, b_sb, Wr_sb = W_sb[l]
                aggS = aggp.tile([P, NT * P], f32, tag="aggS")

                no_gather = bool(os.environ.get("BASSK_NO_GATHER"))
                for (t, j, K, col) in schedule:
                    g = gp.tile([P, KCAP * P], f32, tag="g")
                    if no_gather:
                        nc.sync.dma_start(
                            out=g[:, 0:K * P],
                            in_=table[:].rearrange(
                                "(a b) f -> a (b f)", a=P)[:, 0:K * P])
                    else:
                        for k in range(K):
                            nc.gpsimd.indirect_dma_start(
                                out=g[:, k * P:(k + 1) * P],
                                out_offset=None,
                                in_=table[:],
                                in_offset=bass.IndirectOffsetOnAxis(
                                    ap=idx_sb[:, col + k:col + k + 1], axis=0),
                            )
                    # single-instruction segment sum over K slots (k innermost)
                    dsl = slice(t * P, (t + 1) * P)
                    red = gp.tile([P, P], f32, tag="red")
                    if K > 1:
                        nc.vector.tensor_reduce(
                            out=red[:],
                            in_=g[:, 0:K * P].rearrange("p (k f) -> p f k", k=K),
                            axis=mybir.AxisListType.X, op=AOT.add)
                        rsrc = red
                    else:
                        rsrc = None  # use g directly
                    src_ap = red[:] if rsrc is not None else g[:, 0:P]
                    if j == 0:
                        nc.vector.tensor_tensor(
                            out=aggS[:, dsl], in0=src_ap,
                            in1=invd_sb[:, t:t + 1].to_broadcast([P, P]),
                            op=AOT.mult)
                    else:
                        nc.vector.tensor_tensor(
                            out=red[:] if rsrc is not None else g[:, 0:P],
                            in0=src_ap,
                            in1=invd_sb[:, t:t + 1].to_broadcast([P, P]),
                            op=AOT.mult)
                        nc.vector.tensor_add(aggS[:, dsl], aggS[:, dsl], src_ap)

                # transpose aggS -> aggT (feature-major); alternate copy engine
                aggT = aggp.tile([P, NT * P], f32, tag="aggT")
                for t in range(NT):
                    tp = pst.tile([P, P], f32, tag="tp")
                    nc.tensor.transpose(
                        out=tp[:], in_=aggS[:, t * P:(t + 1) * P], identity=ident[:])
                    if t % 2 == 0:
                        nc.vector.tensor_copy(out=aggT[:, t * P:(t + 1) * P], in_=tp[:])
                    else:
                        nc.scalar.copy(out=aggT[:, t * P:(t + 1) * P], in_=tp[:])

                # matmuls + bias + relu -> next hT
                hT_new = hTp.tile([F, NT * P], f32, tag="hTn")
                for ch in range(NT * P // CH):
                    csl = slice(ch * CH, (ch + 1) * CH)
                    mm = psm.tile([P, CH], f32, tag="mm")
                    nc.tensor.matmul(out=mm[:], lhsT=Wl_sb[:], rhs=aggT[:, csl],
                                     start=True, stop=False)
                    nc.tensor.matmul(out=mm[:], lhsT=Wr_sb[:], rhs=hT_cur[:, csl],
                                     start=False, stop=True)
                    nc.scalar.activation(out=hT_new[:, csl], in_=mm[:],
                                         func=AFT.Relu, bias=b_sb[:, 0:1])

                # transpose back per 128-block: pooling matmul + slice write
                pool_ps = pss.tile([G, F], f32, tag="poolps")
                for t in range(NT):
                    tp = pst.tile([P, P], f32, tag="tp")
                    nc.tensor.transpose(
                        out=tp[:], in_=hT_new[:, t * P:(t + 1) * P], identity=ident[:])
                    hrow = hp.tile([P, F], f32, tag="hrow")
                    if t % 2 == 0:
                        nc.vector.tensor_copy(out=hrow[:], in_=tp[:])
                    else:
                        nc.scalar.copy(out=hrow[:], in_=tp[:])
                    nc.tensor.matmul(out=pool_ps[:], lhsT=B_sb[:, t * G:(t + 1) * G],
                                     rhs=hrow[:], start=(t == 0), stop=(t == NT - 1))
                    if l < 2:
                        r0 = t * P
                        nrows = min(P, S - r0)
                        if nrows > 0:
                            nc.sync.dma_start(out=slice_b[l][r0:r0 + nrows, :],
                                              in_=hrow[:nrows, :])
                nc.vector.tensor_copy(out=pooled[:, l * F:(l + 1) * F], in_=pool_ps[:])

                if l == 0 and os.environ.get("BASSK_DBG"):
                    nc.sync.dma_start(out=dbg[:], in_=slice_b[0][:])
                if l < 2 and not os.environ.get("BASSK_NO_COLL"):
                    nc.gpsimd.collective_compute(
                        "AllGather", AOT.bypass,
                        replica_groups=[list(range(C))],
                        ins=[slice_b[l][:]],
                        outs=[hg[l][:]],
                    )
                hT_cur = hT_new

            # pooled partials -> DRAM -> AllReduce -> back
            for l in range(3):
                nc.sync.dma_start(out=pool_b[l * G:(l + 1) * G, :],
                                  in_=pooled[:, l * F:(l + 1) * F])
            nc.gpsimd.collective_compute(
                "AllReduce", AOT.add, replica_groups=[list(range(C))],
                ins=[pool_b[:]], outs=[pool_r[:]])
            pooledR = pp.tile([G, 3 * F], f32)
            for l in range(3):
                nc.sync.dma_start(out=pooledR[:, l * F:(l + 1) * F],
                                  in_=pool_r[l * G:(l + 1) * G, :])

            # transpose pooled blocks, final matmul, relu, transpose, store
            rhsT = pp.tile([F, 3 * G], f32)
            for l in range(3):
                tp = pss.tile([P, G], f32, tag="tpg")
                nc.tensor.transpose(
                    out=tp[:], in_=pooledR[:, l * F:(l + 1) * F],
                    identity=ident[:G, :G])
                nc.vector.tensor_copy(out=rhsT[:, l * G:(l + 1) * G], in_=tp[:])
            fin = pss.tile([F, G], f32, tag="fin")
            for l in range(3):
                nc.tensor.matmul(out=fin[:], lhsT=wlin_sb[:, l * F:(l + 1) * F],
                                 rhs=rhsT[:, l * G:(l + 1) * G],
                                 start=(l == 0), stop=(l == 2))
            outT = pp.tile([F, G], f32)
            nc.scalar.activation(out=outT[:], in_=fin[:], func=AFT.Relu,
                                 bias=blin_sb[:, 0:1])
            outp = pss.tile([G, F], f32, tag="outp")
            nc.tensor.transpose(out=outp[:], in_=outT[:], identity=ident[:])
            out_sb = pp.tile([G, F], f32)
            nc.vector.tensor_copy(out=out_sb[:], in_=outp[:])
            nc.sync.dma_start(out=out[:], in_=out_sb[:])

    nc.compile()
    return nc


def _make_runner(nc, n_cores):
    import jax
    from jax.sharding import Mesh, PartitionSpec
    from jax.experimental.shard_map import shard_map
    from concourse import mybir
    from concourse.bass2jax import (_bass_exec_p, install_neuronx_cc_hook,
                                    partition_id_tensor)

    install_neuronx_cc_hook()
    partition_name = nc.partition_id_tensor.name if nc.partition_id_tensor else None
    in_names, out_names, out_avals, zero_outs = [], [], [], []
    for alloc in nc.m.functions[0].allocations:
        if not isinstance(alloc, mybir.MemoryLocationSet):
            continue
        name = alloc.memorylocations[0].name
        if alloc.kind == "ExternalInput":
            if name != partition_name:
                in_names.append(name)
        elif alloc.kind == "ExternalOutput":
            out_names.append(name)
            shape = tuple(alloc.tensor_shape)
            dtype = mybir.dt.np(alloc.dtype)
            out_avals.append(jax.core.ShapedArray(shape, dtype))
            zero_outs.append(np.zeros(shape, dtype))
    n_params = len(in_names)
    in_names_all = in_names + out_names
    if partition_name is not None:
        in_names_all = in_names_all + [partition_name]
    dbg_extra = {}
    if nc.dbg_addr is not None:
        dbg_extra[nc.dbg_addr.name] = np.zeros((1, 2), np.uint32)

    def _body(*args):
        operands = list(args)
        if partition_name is not None:
            operands.append(partition_id_tensor())
        outs = _bass_exec_p.bind(
            *operands, out_avals=tuple(out_avals), in_names=tuple(in_names_all),
            out_names=tuple(out_names), lowering_input_output_aliases=(),
            sim_require_finite=True, sim_require_nnan=True, nc=nc)
        return tuple(outs)

    devices = jax.devices()[:n_cores]
    mesh = Mesh(np.asarray(devices), ("core",))
    nspec = (PartitionSpec("core"),) * (n_params + len(out_avals))
    sharded = jax.jit(
        shard_map(_body, mesh=mesh, in_specs=nspec,
                  out_specs=(PartitionSpec("core"),) * len(out_names),
                  check_rep=False),
        keep_unused=True)

    def run(in_maps):
        per_core = [[np.asarray({**m, **dbg_extra}[name]) for name in in_names]
                    for m in in_maps]
        concat_in = [np.concatenate([per_core[c][i] for c in range(n_cores)], axis=0)
                     for i in range(n_params)]
        concat_zeros = [np.zeros((n_cores * z.shape[0], *z.shape[1:]), z.dtype)
                        for z in zero_outs]
        import jax as _jax
        out_arrs = sharded(*concat_in, *concat_zeros)
        _jax.block_until_ready(out_arrs)
        return [
            {name: np.asarray(out_arrs[i]).reshape(n_cores, *out_avals[i].shape)[c]
             for i, name in enumerate(out_names)}
            for c in range(n_cores)
        ], sharded

    return run


_CACHE = {}


def kernel(**inputs):
    x = np.asarray(inputs["x"], dtype=np.float32)
    edge_index = np.asarray(inputs["edge_index"])
    batch = np.asarray(inputs["batch"])

    xp, xsliceT, idx_arr, invdeg, Bmat, schedule0, totk = _host_prep(
        x, edge_index, batch)

    # attach running column offsets to the schedule
    schedule = []
    col = 0
    for (t, j, k) in schedule0:
        schedule.append((t, j, k, col))
        col += k

    key = ("prog", tuple((t, j, k) for t, j, k in schedule0))
    if key not in _CACHE:
        shutil.rmtree(os.path.expanduser("~/.neuron-compile-cache"),
                      ignore_errors=True)
        nc = _build_program(schedule, totk)
        _CACHE[key] = (_make_runner(nc, C), nc)
    run, nc = _CACHE[key]

    Wlin = np.asarray(inputs["Wlin"], dtype=np.float32)
    WlinT = np.ascontiguousarray(
        Wlin.reshape(3, F, F).transpose(1, 0, 2).reshape(F, 3 * F))

    in_maps = []
    for c in range(C):
        m = {
            "xperm": xp,
            "xsliceT": np.ascontiguousarray(xsliceT[c]),
            "idxs": np.ascontiguousarray(idx_arr[c]),
            "invdeg": np.ascontiguousarray(invdeg[c]),
            "Bmat": np.ascontiguousarray(Bmat[c]),
            "WlinT": WlinT,
            "blin": np.asarray(inputs["blin"], dtype=np.float32).reshape(F, 1),
        }
        for l in (1, 2, 3):
            m[f"W{l}l"] = np.asarray(inputs[f"W{l}l"], dtype=np.float32)
            m[f"W{l}r"] = np.asarray(inputs[f"W{l}r"], dtype=np.float32)
            m[f"b{l}"] = np.asarray(inputs[f"b{l}"], dtype=np.float32).reshape(F, 1)
        in_maps.append(m)

    results, _ = run(in_maps)
    kernel._last = (results, schedule, xp, idx_arr, invdeg)
    return results[0]["out"]
